# revision 10
# baseline (speedup 1.0000x reference)
"""Trainium2 Bass kernel for RelPatchAttention3D_TCHW.

Reference computation (B=2, T=8, C=H=W=128, 4x4x4 patches of 32x32x32):
  per batch: n=512 patch tokens, pc=32768 content dims
    qf, kf = patchify(q|k) * pc**-0.5                    [n, pc]
    qk = qf @ kf.T                                       [n, n]
    sim = exp(log(qk + s) - log(qq[:,None] + kk[None,:] - qk + s))
    attn = sim.sum(axis=0)                               [n]   (sum over queries)
    out = attn[:, None] * patchify(v)

Sharding (8 cores): core = 4*b + j handles batch b and KEY-token shard
j (128 of 512 tokens). Each core computes the full-query x key-shard
qk block [512q x 128k] (contraction pc=32768 on partitions, 256
accumulating matmuls into one PSUM bank), applies the log/exp
similarity on-chip (Ln of negative -> NaN exactly as the reference),
reduces over queries locally (free-axis reduce -- no cross-core
reduction needed), and scales its v shard. No collectives.

Precision: q/k are pre-scaled by 128*pc**-0.5 on host and cast to
fp8e4m3 for the GEMM (the power-of-2 factor 2^14 is divided back out
exactly via the activation scale field when reading PSUM); v flows as
bf16. PSUM accumulation is fp32. qq/kk (per-token squared norms) are
computed on host in fp32 from the unquantized data, pre-multiplied by
2^14 (exact) so the on-chip denominator matches reference arithmetic.
"""

import math
import os

import numpy as np
import ml_dtypes

import concourse.bacc as bacc
import concourse.tile as tile
from concourse import mybir
from concourse.bass_utils import run_bass_kernel_spmd

B, T, C, H, W = 2, 8, 128, 128, 128
S = 4
N = T * S * S * S          # 512 patch tokens per batch
PC = (C // S) * (H // S) * (W // S)  # 32768 patch content size
NT = PC // 128             # 256 contraction tiles
NKEY = N // 4              # 128 key tokens per core
SMOOTH = 1e-5
NCORES = 8

FP8_SHIFT = 128.0          # power-of-2 pre-scale for fp8 q/k
PSUM_SCALE = 1.0 / (FP8_SHIFT * FP8_SHIFT)  # 2^-14, exact

QCH = 8                    # q k-tiles per DMA chunk: [128, 8, 512] fp8 = 0.5 MiB
NQCH = NT // QCH
VCH = 4096                 # out chunk: [128, 4096] bf16 = 1 MiB
NVCH = PC // VCH

_BF16 = ml_dtypes.bfloat16
_FP8 = ml_dtypes.float8_e4m3

_nc_cache = {}


def _build_nc(reps=1, external_io=True):
    """Build the SPMD kernel. external_io=False builds a timing variant with
    Internal DRAM tensors (no host transfer) and a tiny dummy output."""
    nc = bacc.Bacc("TRN2", target_bir_lowering=False, debug=False,
                   num_devices=NCORES)

    def dram(name, shape, dt, kind):
        if external_io:
            return nc.dram_tensor(name, shape, dt, kind=kind)
        return nc.dram_tensor(name, shape, dt)

    qT = dram("qT", [128, NT, N], mybir.dt.float8e4, "ExternalInput")
    kT = dram("kT", [128, NT, NKEY], mybir.dt.float8e4, "ExternalInput")
    vv = dram("vv", [128, PC], mybir.dt.bfloat16, "ExternalInput")
    qqs = dram("qqs", [128, N], mybir.dt.float32, "ExternalInput")
    kkc = dram("kkc", [128, 1], mybir.dt.float32, "ExternalInput")
    out = dram("out", [128, PC], mybir.dt.bfloat16, "ExternalOutput")
    if not external_io:
        dummy = nc.dram_tensor("tout", [1, 1], mybir.dt.float32,
                               kind="ExternalOutput")

    with tile.TileContext(nc) as tc:
        with (
            tc.tile_pool(name="const", bufs=1) as const,
            tc.tile_pool(name="qpool", bufs=10) as qpool,
            tc.tile_pool(name="ep", bufs=1) as ep,
            tc.tile_pool(name="psum", bufs=1, space="PSUM") as psum,
        ):
            for _rep in range(reps):
                # k shard resident in SBUF: 4 tiles of [128, 64, 128] fp8
                kt_sb = []
                for s_ in range(4):
                    t_ = const.tile([128, NT // 4, NKEY], mybir.dt.float8e4,
                                    tag=f"kt{s_}")
                    nc.sync.dma_start(
                        out=t_, in_=kT[:, s_ * (NT // 4):(s_ + 1) * (NT // 4), :])
                    kt_sb.append(t_)
                qqs_sb = const.tile([128, N], mybir.dt.float32, tag="qqs")
                nc.sync.dma_start(out=qqs_sb, in_=qqs[:, :])
                kkc_sb = const.tile([128, 1], mybir.dt.float32, tag="kkc")
                nc.sync.dma_start(out=kkc_sb, in_=kkc[:, :])
                smooth_sb = const.tile([128, 1], mybir.dt.float32, tag="smooth")
                nc.vector.memset(smooth_sb, SMOOTH)

                # qk^T block [128 keys, 512 queries] accumulated over 256
                # k-tiles (fp8 DoubleRow: two k-tiles per PE instruction);
                # PSUM holds 2^14 * qk
                pq = psum.tile([128, N], mybir.dt.float32)
                for g in range(NQCH):
                    qch = qpool.tile([128, QCH, N], mybir.dt.float8e4,
                                     tag="qch")
                    nc.sync.dma_start(out=qch,
                                      in_=qT[:, g * QCH:(g + 1) * QCH, :])
                    for i in range(0, QCH, 2):
                        t = g * QCH + i
                        tt = t % (NT // 4)
                        nc.tensor.matmul(
                            pq,
                            lhsT=kt_sb[t // (NT // 4)][:, tt:tt + 2, :],
                            rhs=qch[:, i:i + 2, :],
                            start=(t == 0),
                            stop=(t == NT - 2),
                            perf_mode=mybir.MatmulPerfMode.DoubleRow,
                        )

                # v shard preloaded into SBUF while the GEMM streams.
                # v/out DMAs issue from the ACT HWDGE ring (nc.scalar) while
                # q/k use the SP ring -- measured ~10% faster on HW than
                # funneling all DMAs through one ring.
                v_sb = const.tile([128, PC], mybir.dt.bfloat16, tag="vall")
                for s_ in range(8):
                    sl = slice(s_ * (PC // 8), (s_ + 1) * (PC // 8))
                    nc.scalar.dma_start(out=v_sb[:, sl], in_=vv[:, sl])

                # sim = exp(ln(qk + s) - ln(denum)) with denum = qq+kk-qk+s.
                # Computed as sim = (qk+s)/denum on DVE (same values to ~2
                # ulp); the reference's NaN-for-(qk+s<0) behavior is injected
                # through la = Ln(qk+s) (HW Ln of a negative is NaN) via
                # sim_final = 0*la - (-sim). Using Ln only (no Exp) keeps ACT
                # on a single table set -- the Ln/Exp set switch costs ~2.7us
                # on the attn critical path in this toolchain.
                # (pq = 2^14 qk; qqs/kkc arrive pre-multiplied by 2^14)
                la = ep.tile([128, N], mybir.dt.float32, tag="la")
                nc.scalar.activation(la, pq, mybir.ActivationFunctionType.Ln,
                                     bias=smooth_sb[:, :], scale=PSUM_SCALE)
                nd = ep.tile([128, N], mybir.dt.float32, tag="nd")
                # nd = (2^14 qk - 2^14 kk) - 2^14 (qq + s)  == -2^14 * denum
                nc.vector.scalar_tensor_tensor(
                    nd, in0=pq, scalar=kkc_sb[:, :], in1=qqs_sb,
                    op0=mybir.AluOpType.subtract, op1=mybir.AluOpType.subtract)
                # a2 = 2^14 (qk + s)
                a2 = ep.tile([128, N], mybir.dt.float32, tag="a2")
                nc.vector.tensor_scalar_add(a2, pq,
                                            float(FP8_SHIFT * FP8_SHIFT
                                                  * SMOOTH))
                # r = 1/nd = -2^-14/denum  (accurate DVE reciprocal)
                rr = ep.tile([128, N], mybir.dt.float32, tag="rr")
                nc.vector.reciprocal(rr, nd)
                # s1 = a2 * r = -(qk+s)/denum = -sim
                s1 = ep.tile([128, N], mybir.dt.float32, tag="s1")
                nc.vector.tensor_mul(s1, a2, rr)
                # sim = 0*la - s1  (NaN where la is NaN); attn = sum_q sim
                sim = ep.tile([128, N], mybir.dt.float32, tag="sim")
                attn = ep.tile([128, 1], mybir.dt.float32, tag="attn")
                nc.vector.scalar_tensor_tensor(
                    sim, in0=la, scalar=0.0, in1=s1,
                    op0=mybir.AluOpType.mult, op1=mybir.AluOpType.subtract,
                    accum_out=attn)

                # out = attn * v: in-place per-partition scale (DVE), store
                for u in range(NVCH):
                    sl = slice(u * VCH, (u + 1) * VCH)
                    nc.vector.tensor_scalar_mul(v_sb[:, sl], v_sb[:, sl],
                                                attn[:, :])
                    nc.scalar.dma_start(out=out[:, sl], in_=v_sb[:, sl])
            if not external_io:
                dtile = const.tile([1, 1], mybir.dt.float32, tag="dummy")
                nc.vector.memset(dtile, 1.0)
                nc.sync.dma_start(out=dummy[:, :], in_=dtile)
    nc.compile()
    return nc


def get_nc(reps=1, external_io=True):
    key = (reps, external_io)
    if key not in _nc_cache:
        _nc_cache[key] = _build_nc(reps, external_io)
    return _nc_cache[key]


def _patchify(x):
    # [B,T,C,H,W] -> [B, N, PC] float32, token n = ((t*4+s0)*4+s1)*4+s2
    b = x.shape[0]
    x = x.reshape(b, T, S, C // S, S, H // S, S, W // S)
    x = x.transpose(0, 1, 2, 4, 6, 3, 5, 7)
    return np.ascontiguousarray(x.reshape(b, T * S * S * S, PC))


def _unpatchify(p):
    # [B, N, PC] -> [B,T,C,H,W]
    b = p.shape[0]
    p = p.reshape(b, T, S, S, S, C // S, H // S, W // S)
    p = p.transpose(0, 1, 2, 5, 3, 6, 4, 7)
    return np.ascontiguousarray(p.reshape(b, T, C, H, W))


def prepare_in_maps(q, k, v):
    q = np.asarray(q, dtype=np.float32)
    k = np.asarray(k, dtype=np.float32)
    v = np.asarray(v, dtype=np.float32)
    scale = np.reciprocal(np.sqrt(np.float32(PC)))
    qf = _patchify(q) * scale          # [B, N, PC] f32 (reference arithmetic)
    kf = _patchify(k) * scale
    vf = _patchify(v)

    qq = np.einsum("bnd,bnd->bn", qf, qf, optimize=True)   # [B, N] f32
    kk = np.einsum("bnd,bnd->bn", kf, kf, optimize=True)
    shift = np.float32(FP8_SHIFT)
    shift2 = np.float32(FP8_SHIFT * FP8_SHIFT)

    in_maps = []
    for b in range(B):
        # qT[p, t, nq] = 128 * qf[b, nq, t*128+p]
        qT_b = np.ascontiguousarray(
            (qf[b] * shift).reshape(N, NT, 128).transpose(2, 1, 0)
        ).astype(_FP8)
        qqs_b = np.ascontiguousarray(np.broadcast_to(
            ((qq[b] + np.float32(SMOOTH)) * shift2)[None, :], (128, N)))
        for j in range(4):
            sl = slice(j * NKEY, (j + 1) * NKEY)
            # kT[p, t, m] = 128 * kf[b, j*128+m, t*128+p]
            kT_cj = np.ascontiguousarray(
                (kf[b, sl] * shift).reshape(NKEY, NT, 128).transpose(2, 1, 0)
            ).astype(_FP8)
            in_maps.append({
                "qT": qT_b,
                "kT": kT_cj,
                "vv": vf[b, sl].astype(_BF16),
                "qqs": qqs_b,
                "kkc": np.ascontiguousarray(
                    (kk[b, sl] * shift2).reshape(NKEY, 1)),
            })
    return in_maps


def assemble_output(results, dtype=np.float32):
    outf = np.empty((B, N, PC), dtype=np.float32)
    for b in range(B):
        for j in range(4):
            outf[b, j * NKEY:(j + 1) * NKEY, :] = \
                results[4 * b + j]["out"].astype(np.float32)
    return _unpatchify(outf).astype(dtype, copy=False)


def kernel(q, k, v):
    in_dtype = np.asarray(q).dtype
    in_maps = prepare_in_maps(q, k, v)
    nc = get_nc(reps=1)
    res = run_bass_kernel_spmd(nc, in_maps, core_ids=list(range(NCORES)))
    return assemble_output(res.results, dtype=in_dtype)


# revision 13
# speedup vs baseline: 1.0196x; 1.0196x over previous
"""Trainium2 Bass kernel for RelPatchAttention3D_TCHW.

Reference computation (B=2, T=8, C=H=W=128, 4x4x4 patches of 32x32x32):
  per batch: n=512 patch tokens, pc=32768 content dims
    qf, kf = patchify(q|k) * pc**-0.5                    [n, pc]
    qk = qf @ kf.T                                       [n, n]
    sim = exp(log(qk + s) - log(qq[:,None] + kk[None,:] - qk + s))
    attn = sim.sum(axis=0)                               [n]   (sum over queries)
    out = attn[:, None] * patchify(v)

Sharding (8 cores): core = 4*b + j handles batch b and KEY-token shard
j (128 of 512 tokens). Each core computes the full-query x key-shard
qk block [512q x 128k] (contraction pc=32768 on partitions, 256
accumulating matmuls into one PSUM bank), applies the log/exp
similarity on-chip (Ln of negative -> NaN exactly as the reference),
reduces over queries locally (free-axis reduce -- no cross-core
reduction needed), and scales its v shard. No collectives.

Precision: q/k are pre-scaled by 128*pc**-0.5 on host and cast to
fp8e4m3 for the GEMM (the power-of-2 factor 2^14 is divided back out
exactly via the activation scale field when reading PSUM); v flows as
bf16. PSUM accumulation is fp32. qq/kk (per-token squared norms) are
computed on host in fp32 from the unquantized data, pre-multiplied by
2^14 (exact) so the on-chip denominator matches reference arithmetic.
"""

import math
import os

import numpy as np
import ml_dtypes

import concourse.bacc as bacc
import concourse.tile as tile
from concourse import mybir
from concourse.bass_utils import run_bass_kernel_spmd

B, T, C, H, W = 2, 8, 128, 128, 128
S = 4
N = T * S * S * S          # 512 patch tokens per batch
PC = (C // S) * (H // S) * (W // S)  # 32768 patch content size
NT = PC // 128             # 256 contraction tiles
NKEY = N // 4              # 128 key tokens per core
SMOOTH = 1e-5
NCORES = 8

FP8_SHIFT = 128.0          # power-of-2 pre-scale for fp8 q/k
PSUM_SCALE = 1.0 / (FP8_SHIFT * FP8_SHIFT)  # 2^-14, exact

QCH = 8                    # q k-tiles per DMA chunk: [128, 8, 512] fp8 = 0.5 MiB
NQCH = NT // QCH
VCH = 4096                 # out chunk: [128, 4096] bf16 = 1 MiB
NVCH = PC // VCH

_BF16 = ml_dtypes.bfloat16
_FP8 = ml_dtypes.float8_e4m3

_nc_cache = {}


def _build_nc(reps=1, external_io=True):
    """Build the SPMD kernel. external_io=False builds a timing variant with
    Internal DRAM tensors (no host transfer) and a tiny dummy output."""
    nc = bacc.Bacc("TRN2", target_bir_lowering=False, debug=False,
                   num_devices=NCORES)

    def dram(name, shape, dt, kind):
        if external_io:
            return nc.dram_tensor(name, shape, dt, kind=kind)
        return nc.dram_tensor(name, shape, dt)

    qT = dram("qT", [128, NT, N], mybir.dt.float8e4, "ExternalInput")
    kT = dram("kT", [128, NT, NKEY], mybir.dt.float8e4, "ExternalInput")
    vv = dram("vv", [128, PC], mybir.dt.bfloat16, "ExternalInput")
    qqs = dram("qqs", [128, N], mybir.dt.float32, "ExternalInput")
    kkc = dram("kkc", [128, 1], mybir.dt.float32, "ExternalInput")
    out = dram("out", [128, PC], mybir.dt.bfloat16, "ExternalOutput")
    if not external_io:
        dummy = nc.dram_tensor("tout", [1, 1], mybir.dt.float32,
                               kind="ExternalOutput")

    with tile.TileContext(nc) as tc:
        with (
            tc.tile_pool(name="const", bufs=1) as const,
            tc.tile_pool(name="qpool", bufs=10) as qpool,
            tc.tile_pool(name="ep", bufs=1) as ep,
            tc.tile_pool(name="psum", bufs=1, space="PSUM") as psum,
        ):
            for _rep in range(reps):
                # k shard resident in SBUF: 4 tiles of [128, 64, 128] fp8
                kt_sb = []
                for s_ in range(4):
                    t_ = const.tile([128, NT // 4, NKEY], mybir.dt.float8e4,
                                    tag=f"kt{s_}")
                    nc.sync.dma_start(
                        out=t_, in_=kT[:, s_ * (NT // 4):(s_ + 1) * (NT // 4), :])
                    kt_sb.append(t_)
                qqs_sb = const.tile([128, N], mybir.dt.float32, tag="qqs")
                nc.sync.dma_start(out=qqs_sb, in_=qqs[:, :])
                kkc_sb = const.tile([128, 1], mybir.dt.float32, tag="kkc")
                nc.sync.dma_start(out=kkc_sb, in_=kkc[:, :])
                smooth_sb = const.tile([128, 1], mybir.dt.float32, tag="smooth")
                nc.vector.memset(smooth_sb, SMOOTH)
                smooth14_sb = const.tile([128, 1], mybir.dt.float32,
                                         tag="smooth14")
                nc.vector.memset(smooth14_sb,
                                 float(FP8_SHIFT * FP8_SHIFT * SMOOTH))

                # qk^T block [128 keys, 512 queries] accumulated over 256
                # k-tiles (fp8 DoubleRow: two k-tiles per PE instruction);
                # PSUM holds 2^14 * qk
                pq = psum.tile([128, N], mybir.dt.float32)
                last_q = None
                for g in range(NQCH):
                    qch = qpool.tile([128, QCH, N], mybir.dt.float8e4,
                                     tag="qch")
                    qdma = nc.sync.dma_start(out=qch,
                                             in_=qT[:, g * QCH:(g + 1) * QCH, :])
                    if g == NQCH - 1:
                        last_q = qdma
                    for i in range(0, QCH, 2):
                        t = g * QCH + i
                        tt = t % (NT // 4)
                        nc.tensor.matmul(
                            pq,
                            lhsT=kt_sb[t // (NT // 4)][:, tt:tt + 2, :],
                            rhs=qch[:, i:i + 2, :],
                            start=(t == 0),
                            stop=(t == NT - 2),
                            perf_mode=mybir.MatmulPerfMode.DoubleRow,
                        )

                # v shard preloaded into SBUF while the GEMM streams.
                # v/out DMAs issue from the ACT HWDGE ring (nc.scalar) while
                # q/k use the SP ring -- measured ~10% faster on HW than
                # funneling all DMAs through one ring. The last 4 MiB of v
                # are explicitly held until the q stream completes so their
                # loads fill the otherwise-idle DMA window while the attn
                # epilogue runs (model: -6us single-shot).
                V_HOLD = 4
                v_sb = const.tile([128, PC], mybir.dt.bfloat16, tag="vall")
                for s_ in range(8):
                    sl = slice(s_ * (PC // 8), (s_ + 1) * (PC // 8))
                    vdma = nc.scalar.dma_start(out=v_sb[:, sl], in_=vv[:, sl])
                    if s_ >= 8 - V_HOLD:
                        tile.add_dep_helper(
                            vdma.ins, last_q.ins, sync=True,
                            reason="defer v tail into attn-latency DMA gap")

                # sim = exp(ln(qk + s) - ln(denum)) with denum = qq+kk-qk+s.
                # Computed as sim = (qk+s)/denum on DVE (same values to ~2
                # ulp); the reference's NaN-for-(qk+s<0) behavior is injected
                # through la = Ln(qk+s) (HW Ln of a negative is NaN) via
                # sim_final = 0*la - (-sim). Using Ln only (no Exp) keeps ACT
                # on a single table set -- the Ln/Exp set switch costs ~2.7us
                # on the attn critical path in this toolchain.
                # (pq = 2^14 qk; qqs/kkc arrive pre-multiplied by 2^14)
                # a2 = 2^14 (qk + s) on ACT (runs parallel to DVE nd/rr);
                # smooth14_sb holds 2^14 * SMOOTH
                a2 = ep.tile([128, N], mybir.dt.float32, tag="a2")
                nc.scalar.activation(a2, pq,
                                     mybir.ActivationFunctionType.Identity,
                                     bias=smooth14_sb[:, :])
                la = ep.tile([128, N], mybir.dt.float32, tag="la")
                nc.scalar.activation(la, pq, mybir.ActivationFunctionType.Ln,
                                     bias=smooth_sb[:, :], scale=PSUM_SCALE)
                nd = ep.tile([128, N], mybir.dt.float32, tag="nd")
                # nd = (2^14 qk - 2^14 kk) - 2^14 (qq + s)  == -2^14 * denum
                nc.vector.scalar_tensor_tensor(
                    nd, in0=pq, scalar=kkc_sb[:, :], in1=qqs_sb,
                    op0=mybir.AluOpType.subtract, op1=mybir.AluOpType.subtract)
                # r = 1/nd = -2^-14/denum  (accurate DVE reciprocal)
                rr = ep.tile([128, N], mybir.dt.float32, tag="rr")
                nc.vector.reciprocal(rr, nd)
                # s1 = a2 * r = -(qk+s)/denum = -sim
                s1 = ep.tile([128, N], mybir.dt.float32, tag="s1")
                nc.vector.tensor_mul(s1, a2, rr)
                # sim = 0*la - s1  (NaN where la is NaN); attn = sum_q sim
                sim = ep.tile([128, N], mybir.dt.float32, tag="sim")
                attn = ep.tile([128, 1], mybir.dt.float32, tag="attn")
                nc.vector.scalar_tensor_tensor(
                    sim, in0=la, scalar=0.0, in1=s1,
                    op0=mybir.AluOpType.mult, op1=mybir.AluOpType.subtract,
                    accum_out=attn)

                # out = attn * v: in-place per-partition scale (DVE), store
                for u in range(NVCH):
                    sl = slice(u * VCH, (u + 1) * VCH)
                    nc.vector.tensor_scalar_mul(v_sb[:, sl], v_sb[:, sl],
                                                attn[:, :])
                    nc.scalar.dma_start(out=out[:, sl], in_=v_sb[:, sl])
            if not external_io:
                dtile = const.tile([1, 1], mybir.dt.float32, tag="dummy")
                nc.vector.memset(dtile, 1.0)
                nc.sync.dma_start(out=dummy[:, :], in_=dtile)
    nc.compile()
    return nc


def get_nc(reps=1, external_io=True):
    key = (reps, external_io)
    if key not in _nc_cache:
        _nc_cache[key] = _build_nc(reps, external_io)
    return _nc_cache[key]


def _patchify(x):
    # [B,T,C,H,W] -> [B, N, PC] float32, token n = ((t*4+s0)*4+s1)*4+s2
    b = x.shape[0]
    x = x.reshape(b, T, S, C // S, S, H // S, S, W // S)
    x = x.transpose(0, 1, 2, 4, 6, 3, 5, 7)
    return np.ascontiguousarray(x.reshape(b, T * S * S * S, PC))


def _unpatchify(p):
    # [B, N, PC] -> [B,T,C,H,W]
    b = p.shape[0]
    p = p.reshape(b, T, S, S, S, C // S, H // S, W // S)
    p = p.transpose(0, 1, 2, 5, 3, 6, 4, 7)
    return np.ascontiguousarray(p.reshape(b, T, C, H, W))


def prepare_in_maps(q, k, v):
    q = np.asarray(q, dtype=np.float32)
    k = np.asarray(k, dtype=np.float32)
    v = np.asarray(v, dtype=np.float32)
    scale = np.reciprocal(np.sqrt(np.float32(PC)))
    qf = _patchify(q) * scale          # [B, N, PC] f32 (reference arithmetic)
    kf = _patchify(k) * scale
    vf = _patchify(v)

    qq = np.einsum("bnd,bnd->bn", qf, qf, optimize=True)   # [B, N] f32
    kk = np.einsum("bnd,bnd->bn", kf, kf, optimize=True)
    shift = np.float32(FP8_SHIFT)
    shift2 = np.float32(FP8_SHIFT * FP8_SHIFT)

    in_maps = []
    for b in range(B):
        # qT[p, t, nq] = 128 * qf[b, nq, t*128+p]
        qT_b = np.ascontiguousarray(
            (qf[b] * shift).reshape(N, NT, 128).transpose(2, 1, 0)
        ).astype(_FP8)
        qqs_b = np.ascontiguousarray(np.broadcast_to(
            ((qq[b] + np.float32(SMOOTH)) * shift2)[None, :], (128, N)))
        for j in range(4):
            sl = slice(j * NKEY, (j + 1) * NKEY)
            # kT[p, t, m] = 128 * kf[b, j*128+m, t*128+p]
            kT_cj = np.ascontiguousarray(
                (kf[b, sl] * shift).reshape(NKEY, NT, 128).transpose(2, 1, 0)
            ).astype(_FP8)
            in_maps.append({
                "qT": qT_b,
                "kT": kT_cj,
                "vv": vf[b, sl].astype(_BF16),
                "qqs": qqs_b,
                "kkc": np.ascontiguousarray(
                    (kk[b, sl] * shift2).reshape(NKEY, 1)),
            })
    return in_maps


def assemble_output(results, dtype=np.float32):
    outf = np.empty((B, N, PC), dtype=np.float32)
    for b in range(B):
        for j in range(4):
            outf[b, j * NKEY:(j + 1) * NKEY, :] = \
                results[4 * b + j]["out"].astype(np.float32)
    return _unpatchify(outf).astype(dtype, copy=False)


def kernel(q, k, v):
    in_dtype = np.asarray(q).dtype
    in_maps = prepare_in_maps(q, k, v)
    nc = get_nc(reps=1)
    res = run_bass_kernel_spmd(nc, in_maps, core_ids=list(range(NCORES)))
    return assemble_output(res.results, dtype=in_dtype)


# revision 14
# speedup vs baseline: 1.1784x; 1.1558x over previous
"""Trainium2 Bass kernel for RelPatchAttention3D_TCHW.

Reference computation (B=2, T=8, C=H=W=128, 4x4x4 patches of 32x32x32):
  per batch: n=512 patch tokens, pc=32768 content dims
    qf, kf = patchify(q|k) * pc**-0.5                    [n, pc]
    qk = qf @ kf.T                                       [n, n]
    sim = exp(log(qk + s) - log(qq[:,None] + kk[None,:] - qk + s))
    attn = sim.sum(axis=0)                               [n]   (sum over queries)
    out = attn[:, None] * patchify(v)

Sharding (8 cores): core = 4*b + j handles batch b and KEY-token shard
j (128 of 512 tokens). Each core computes the full-query x key-shard
qk block [512q x 128k] (contraction pc=32768 on partitions, 256
accumulating matmuls into one PSUM bank), applies the log/exp
similarity on-chip (Ln of negative -> NaN exactly as the reference),
reduces over queries locally (free-axis reduce -- no cross-core
reduction needed), and scales its v shard. No collectives.

Precision: q/k are pre-scaled by 128*pc**-0.5 on host and cast to
fp8e4m3 for the GEMM (the power-of-2 factor 2^14 is divided back out
exactly via the activation scale field when reading PSUM); v flows as
bf16. PSUM accumulation is fp32. qq/kk (per-token squared norms) are
computed on host in fp32 from the unquantized data, pre-multiplied by
2^14 (exact) so the on-chip denominator matches reference arithmetic.
"""

import math
import os

import numpy as np
import ml_dtypes

import concourse.bacc as bacc
import concourse.tile as tile
from concourse import mybir
from concourse.bass_utils import run_bass_kernel_spmd

B, T, C, H, W = 2, 8, 128, 128, 128
S = 4
N = T * S * S * S          # 512 patch tokens per batch
PC = (C // S) * (H // S) * (W // S)  # 32768 patch content size
NT = PC // 128             # 256 contraction tiles
NKEY = N // 4              # 128 key tokens per core
SMOOTH = 1e-5
NCORES = 8

FP8_SHIFT = 128.0          # power-of-2 pre-scale for fp8 q/k
PSUM_SCALE = 1.0 / (FP8_SHIFT * FP8_SHIFT)  # 2^-14, exact

QCH = 8                    # q k-tiles per DMA chunk: [128, 8, 512] fp8 = 0.5 MiB
NQCH = NT // QCH
VCH = 4096                 # out chunk: [128, 4096] bf16 = 1 MiB
NVCH = PC // VCH

_BF16 = ml_dtypes.bfloat16
_FP8 = ml_dtypes.float8_e4m3

_nc_cache = {}


def _build_nc(reps=1, external_io=True):
    """Build the SPMD kernel. external_io=False builds a timing variant with
    Internal DRAM tensors (no host transfer) and a tiny dummy output."""
    nc = bacc.Bacc("TRN2", target_bir_lowering=False, debug=False,
                   num_devices=NCORES)

    def dram(name, shape, dt, kind):
        if external_io:
            return nc.dram_tensor(name, shape, dt, kind=kind)
        return nc.dram_tensor(name, shape, dt)

    qT = dram("qT", [128, NT, N], mybir.dt.float8e4, "ExternalInput")
    kT = dram("kT", [128, NT, NKEY], mybir.dt.float8e4, "ExternalInput")
    vv = dram("vv", [128, PC], mybir.dt.bfloat16, "ExternalInput")
    qqs = dram("qqs", [128, N], mybir.dt.float32, "ExternalInput")
    kkc = dram("kkc", [128, 1], mybir.dt.float32, "ExternalInput")
    out = dram("out", [128, PC], mybir.dt.bfloat16, "ExternalOutput")
    if not external_io:
        dummy = nc.dram_tensor("tout", [1, 1], mybir.dt.float32,
                               kind="ExternalOutput")

    with tile.TileContext(nc) as tc:
        with (
            tc.tile_pool(name="const", bufs=1) as const,
            tc.tile_pool(name="qpool", bufs=10) as qpool,
            tc.tile_pool(name="ep", bufs=1) as ep,
            tc.tile_pool(name="psum", bufs=1, space="PSUM") as psum,
        ):
            for _rep in range(reps):
                # k shard resident in SBUF: 4 tiles of [128, 64, 128] fp8
                kt_sb = []
                for s_ in range(4):
                    t_ = const.tile([128, NT // 4, NKEY], mybir.dt.float8e4,
                                    tag=f"kt{s_}")
                    nc.sync.dma_start(
                        out=t_, in_=kT[:, s_ * (NT // 4):(s_ + 1) * (NT // 4), :])
                    kt_sb.append(t_)
                qqs_sb = const.tile([128, N], mybir.dt.float32, tag="qqs")
                nc.sync.dma_start(out=qqs_sb, in_=qqs[:, :])
                kkc_sb = const.tile([128, 1], mybir.dt.float32, tag="kkc")
                nc.sync.dma_start(out=kkc_sb, in_=kkc[:, :])
                smooth_sb = const.tile([128, 1], mybir.dt.float32, tag="smooth")
                nc.vector.memset(smooth_sb, SMOOTH)
                smooth14_sb = const.tile([128, 1], mybir.dt.float32,
                                         tag="smooth14")
                nc.vector.memset(smooth14_sb,
                                 float(FP8_SHIFT * FP8_SHIFT * SMOOTH))

                # qk^T block [128 keys, 512 queries] accumulated over 256
                # k-tiles (fp8 DoubleRow: two k-tiles per PE instruction);
                # PSUM holds 2^14 * qk
                pq = psum.tile([128, N], mybir.dt.float32)
                last_q = None
                for g in range(NQCH):
                    qch = qpool.tile([128, QCH, N], mybir.dt.float8e4,
                                     tag="qch")
                    qdma = nc.sync.dma_start(out=qch,
                                             in_=qT[:, g * QCH:(g + 1) * QCH, :])
                    if g == NQCH - 1:
                        last_q = qdma
                    for i in range(0, QCH, 2):
                        t = g * QCH + i
                        tt = t % (NT // 4)
                        nc.tensor.matmul(
                            pq,
                            lhsT=kt_sb[t // (NT // 4)][:, tt:tt + 2, :],
                            rhs=qch[:, i:i + 2, :],
                            start=(t == 0),
                            stop=(t == NT - 2),
                            perf_mode=mybir.MatmulPerfMode.DoubleRow,
                        )

                # v shard preloaded into SBUF while the GEMM streams.
                # v/out DMAs issue from the ACT HWDGE ring (nc.scalar) while
                # q/k use the SP ring -- measured ~10% faster on HW than
                # funneling all DMAs through one ring. The last 4 MiB of v
                # are explicitly held until the q stream completes so their
                # loads fill the otherwise-idle DMA window while the attn
                # epilogue runs (model: -6us single-shot).
                V_HOLD = 4
                v_sb = const.tile([128, PC], mybir.dt.bfloat16, tag="vall")
                for s_ in range(8):
                    sl = slice(s_ * (PC // 8), (s_ + 1) * (PC // 8))
                    # held chunks go on the SP ring (idle once q completes)
                    # so out-stores on the ACT ring don't queue behind them
                    eng = nc.sync if s_ >= 8 - V_HOLD else nc.scalar
                    vdma = eng.dma_start(out=v_sb[:, sl], in_=vv[:, sl])
                    if s_ >= 8 - V_HOLD:
                        tile.add_dep_helper(
                            vdma.ins, last_q.ins, sync=True,
                            reason="defer v tail into attn-latency DMA gap")

                # sim = exp(ln(qk + s) - ln(denum)) with denum = qq+kk-qk+s.
                # Computed as sim = (qk+s)/denum on DVE (same values to ~2
                # ulp); the reference's NaN-for-(qk+s<0) behavior is injected
                # through la = Ln(qk+s) (HW Ln of a negative is NaN) via
                # sim_final = 0*la - (-sim). Using Ln only (no Exp) keeps ACT
                # on a single table set -- the Ln/Exp set switch costs ~2.7us
                # on the attn critical path in this toolchain.
                # (pq = 2^14 qk; qqs/kkc arrive pre-multiplied by 2^14)
                # a2 = 2^14 (qk + s) on ACT (runs parallel to DVE nd/rr);
                # smooth14_sb holds 2^14 * SMOOTH
                a2 = ep.tile([128, N], mybir.dt.float32, tag="a2")
                nc.scalar.activation(a2, pq,
                                     mybir.ActivationFunctionType.Identity,
                                     bias=smooth14_sb[:, :])
                la = ep.tile([128, N], mybir.dt.float32, tag="la")
                nc.scalar.activation(la, pq, mybir.ActivationFunctionType.Ln,
                                     bias=smooth_sb[:, :], scale=PSUM_SCALE)
                nd = ep.tile([128, N], mybir.dt.float32, tag="nd")
                # nd = (2^14 qk - 2^14 kk) - 2^14 (qq + s)  == -2^14 * denum
                nc.vector.scalar_tensor_tensor(
                    nd, in0=pq, scalar=kkc_sb[:, :], in1=qqs_sb,
                    op0=mybir.AluOpType.subtract, op1=mybir.AluOpType.subtract)
                # r = 1/nd = -2^-14/denum  (accurate DVE reciprocal)
                rr = ep.tile([128, N], mybir.dt.float32, tag="rr")
                nc.vector.reciprocal(rr, nd)
                # s1 = a2 * r = -(qk+s)/denum = -sim
                s1 = ep.tile([128, N], mybir.dt.float32, tag="s1")
                nc.vector.tensor_mul(s1, a2, rr)
                # sim = 0*la - s1  (NaN where la is NaN); attn = sum_q sim
                sim = ep.tile([128, N], mybir.dt.float32, tag="sim")
                attn = ep.tile([128, 1], mybir.dt.float32, tag="attn")
                nc.vector.scalar_tensor_tensor(
                    sim, in0=la, scalar=0.0, in1=s1,
                    op0=mybir.AluOpType.mult, op1=mybir.AluOpType.subtract,
                    accum_out=attn)

                # out = attn * v: in-place per-partition scale (DVE), store
                for u in range(NVCH):
                    sl = slice(u * VCH, (u + 1) * VCH)
                    nc.vector.tensor_scalar_mul(v_sb[:, sl], v_sb[:, sl],
                                                attn[:, :])
                    nc.scalar.dma_start(out=out[:, sl], in_=v_sb[:, sl])
            if not external_io:
                dtile = const.tile([1, 1], mybir.dt.float32, tag="dummy")
                nc.vector.memset(dtile, 1.0)
                nc.sync.dma_start(out=dummy[:, :], in_=dtile)
    nc.compile()
    return nc


def get_nc(reps=1, external_io=True):
    key = (reps, external_io)
    if key not in _nc_cache:
        _nc_cache[key] = _build_nc(reps, external_io)
    return _nc_cache[key]


def _patchify(x):
    # [B,T,C,H,W] -> [B, N, PC] float32, token n = ((t*4+s0)*4+s1)*4+s2
    b = x.shape[0]
    x = x.reshape(b, T, S, C // S, S, H // S, S, W // S)
    x = x.transpose(0, 1, 2, 4, 6, 3, 5, 7)
    return np.ascontiguousarray(x.reshape(b, T * S * S * S, PC))


def _unpatchify(p):
    # [B, N, PC] -> [B,T,C,H,W]
    b = p.shape[0]
    p = p.reshape(b, T, S, S, S, C // S, H // S, W // S)
    p = p.transpose(0, 1, 2, 5, 3, 6, 4, 7)
    return np.ascontiguousarray(p.reshape(b, T, C, H, W))


def prepare_in_maps(q, k, v):
    q = np.asarray(q, dtype=np.float32)
    k = np.asarray(k, dtype=np.float32)
    v = np.asarray(v, dtype=np.float32)
    scale = np.reciprocal(np.sqrt(np.float32(PC)))
    qf = _patchify(q) * scale          # [B, N, PC] f32 (reference arithmetic)
    kf = _patchify(k) * scale
    vf = _patchify(v)

    qq = np.einsum("bnd,bnd->bn", qf, qf, optimize=True)   # [B, N] f32
    kk = np.einsum("bnd,bnd->bn", kf, kf, optimize=True)
    shift = np.float32(FP8_SHIFT)
    shift2 = np.float32(FP8_SHIFT * FP8_SHIFT)

    in_maps = []
    for b in range(B):
        # qT[p, t, nq] = 128 * qf[b, nq, t*128+p]
        qT_b = np.ascontiguousarray(
            (qf[b] * shift).reshape(N, NT, 128).transpose(2, 1, 0)
        ).astype(_FP8)
        qqs_b = np.ascontiguousarray(np.broadcast_to(
            ((qq[b] + np.float32(SMOOTH)) * shift2)[None, :], (128, N)))
        for j in range(4):
            sl = slice(j * NKEY, (j + 1) * NKEY)
            # kT[p, t, m] = 128 * kf[b, j*128+m, t*128+p]
            kT_cj = np.ascontiguousarray(
                (kf[b, sl] * shift).reshape(NKEY, NT, 128).transpose(2, 1, 0)
            ).astype(_FP8)
            in_maps.append({
                "qT": qT_b,
                "kT": kT_cj,
                "vv": vf[b, sl].astype(_BF16),
                "qqs": qqs_b,
                "kkc": np.ascontiguousarray(
                    (kk[b, sl] * shift2).reshape(NKEY, 1)),
            })
    return in_maps


def assemble_output(results, dtype=np.float32):
    outf = np.empty((B, N, PC), dtype=np.float32)
    for b in range(B):
        for j in range(4):
            outf[b, j * NKEY:(j + 1) * NKEY, :] = \
                results[4 * b + j]["out"].astype(np.float32)
    return _unpatchify(outf).astype(dtype, copy=False)


def kernel(q, k, v):
    in_dtype = np.asarray(q).dtype
    in_maps = prepare_in_maps(q, k, v)
    nc = get_nc(reps=1)
    res = run_bass_kernel_spmd(nc, in_maps, core_ids=list(range(NCORES)))
    return assemble_output(res.results, dtype=in_dtype)
